# revision 5
# baseline (speedup 1.0000x reference)
"""Burgers PDE RHS kernel for Trainium2 (8 NeuronCores, SPMD).

Reference math (per element i of the padded array U, DX = 0.01):
  delta = (l - 2c + r) / DX^2
  adv   = max(c,0)*(c-l)/DX + min(c,0)*(r-c)/DX
  out   = d*delta - adv,  d = sigmoid(d_org)*0.01

Exact reformulation in y-space (y = x/sqrt(DX) = 10x, host pre-scale):
  W = c - r;  D = l + r - 2c;  out = D*(relu(c) + beta) + c*W
  beta = d / DX^1.5 = d*1000.

I/O is fp16 (the rel-err budget is 2e-2; fp16 I/O costs ~2e-4), halving
HBM traffic vs fp32: 16.78 MB per core -> ~47 us at the ~358 GB/s
per-core HBM roofline.

Compute is ONE custom DVE pass per element (1 elem/cycle/partition,
fp32 internal): a single-input-stream 8-slice uop using two chained
swap-flop temporal delays (HW-verified: SUBTRACT with swap_enable
latches operand B):
  slice0: W = swap0 - s (swap0 <- s)      => W_i = s[i-1] - s[i] = c - r
  slice1: D = swap1 - W (swap1 <- W)      => D_i = W[i-1] - W_i = l+r-2c
  slice2: c = W + s
  slice3..7: relu, +beta, *D, c*W, sum
First 2 output elements of each instruction are stale-swap garbage and
are not stored; tiles overlap by 2 input elements to compensate.

Distribution: spatial dim sharded 8 ways; each core gets its S+2 element
slice (halos resolved on host from bc / neighboring shards). On-chip
layout is row-major [128 partitions x 32768 elems]; loads go on the SP
HWDGE ring, stores on the ACT HWDGE ring (separate FIFOs).
"""

import os
import sys

import numpy as np

for _p in ("/opt/trn_rl_repo", "/root/.axon_site/_ro/trn_rl_repo"):
    if _p not in sys.path and os.path.isdir(_p):
        sys.path.append(_p)

import concourse.bacc as bacc
import concourse.mybir as mybir
from concourse.ap import AP
from concourse.bass_utils import run_bass_kernel_spmd
from concourse.tile import TileContext

N_CORES = 8
N_TOTAL = 33554432
S = N_TOTAL // N_CORES          # 4194304 elements per core
P = 128
R = S // P                      # 32768 elements per partition row
DX = 0.01

# ---------------------------------------------------------------------------
# Custom DVE op (registered once, idempotent)
# ---------------------------------------------------------------------------

def _register_ops():
    from concourse.dve_ops import DveOp, OPS, CUSTOM_DVE_SPECS, \
        _SUB_OPCODE_FOR_NAME, _CUSTOM_DVE_ROW_BASE, _COMPILE_CACHE
    from concourse.dve_spec import Spec, Src0, C0, relu
    from concourse.dve_uop import (
        AluInp, AluOp, DelayInp, DveOpSpec, InpSel, OutPath, OutSel, Trigger,
        UopConfig, ENABLE,
    )

    def _burgers_uop():
        u = UopConfig()
        u.enable_input(InpSel.SRC_0, 0)     # stream s -> block0 ALU operand B
        u.enable_input(InpSel.SRC_0, 1)     # s on delay lane 0 (r for c=W+r)
        u.enable_input(InpSel.ZERO, 2)      # 0 on delay lane 1 (for relu)
        u.enable_input(InpSel.CONST_0, 3)   # beta on delay lane 2
        u.require_inp0 = ENABLE
        u.trigger = (Trigger.SRC_TENSOR_DONE, Trigger.NONE, Trigger.NONE)
        u.next_uop = (0, 0, 0)
        u.enable_output(OutSel.ALU_OUT, OutPath.WR0_LO)

        b = u.datapath_config
        # b0: W = swap0 - s  (swap0 <- s)
        b[0].enable_alu(AluOp.SUBTRACT, AluInp.CURR_SWAP_OUT,
                        AluInp.PREV_ALU_OUT)
        b[0].swap_enable = ENABLE
        b[0].pass_through_delay(0, 1, 2)
        # b1: D = swap1 - W  (swap1 <- W)
        b[1].enable_alu(AluOp.SUBTRACT, AluInp.CURR_SWAP_OUT,
                        AluInp.PREV_ALU_OUT)
        b[1].swap_enable = ENABLE
        b[1].enable_delay_from_src(DelayInp.PREV_ALU_OUT, 3)   # lane3 <- W
        b[1].pass_through_delay(0, 1, 2)
        # b2: c = W + r
        b[2].enable_alu(AluOp.ADD, AluInp.PREV_DELAY_3, AluInp.PREV_DELAY_0)
        b[2].enable_delay_from_src(DelayInp.PREV_ALU_OUT, 4)   # lane4 <- D
        b[2].pass_through_delay(1, 2, 3)
        # b3: p = max(c, 0)
        b[3].enable_alu(AluOp.MAX, AluInp.PREV_ALU_OUT, AluInp.PREV_DELAY_1)
        b[3].enable_delay_from_src(DelayInp.PREV_ALU_OUT, 0)   # lane0 <- c
        b[3].pass_through_delay(2, 3, 4)
        # b4: g = p + beta
        b[4].enable_alu(AluOp.ADD, AluInp.PREV_ALU_OUT, AluInp.PREV_DELAY_2)
        b[4].pass_through_delay(0, 3, 4)
        # b5: M = g * D
        b[5].enable_alu(AluOp.MULTIPLY, AluInp.PREV_ALU_OUT,
                        AluInp.PREV_DELAY_4)
        b[5].pass_through_delay(0, 3)
        # b6: Z = c * W
        b[6].enable_alu(AluOp.MULTIPLY, AluInp.PREV_DELAY_0,
                        AluInp.PREV_DELAY_3)
        b[6].enable_delay_from_src(DelayInp.PREV_ALU_OUT, 1)   # lane1 <- M
        # b7: out = Z + M
        b[7].enable_alu(AluOp.ADD, AluInp.PREV_ALU_OUT, AluInp.PREV_DELAY_1)
        u.validate("v3")
        return u

    def _burgers_ref(in0, in1, s0, s1, imm2):
        s = in0
        W = np.empty_like(s)
        W[:, 0] = 0.0 - s[:, 0]
        W[:, 1:] = s[:, :-1] - s[:, 1:]
        D = np.empty_like(s)
        D[:, 0] = 0.0 - W[:, 0]
        D[:, 1:] = W[:, :-1] - W[:, 1:]
        c = W + s
        return D * (np.maximum(c, 0) + s0) + c * W

    class HandDveOp(DveOp):
        """DveOp whose table program is hand-written (bypasses lower())."""

        def __init__(self, name, fake_spec, uops, rd1):
            object.__setattr__(self, "name", name)
            object.__setattr__(self, "spec", fake_spec)
            object.__setattr__(self, "subdim", False)
            object.__setattr__(self, "uops_sha", {})
            object.__setattr__(self, "perf_en", {})
            object.__setattr__(self, "_uops", uops)
            object.__setattr__(self, "_rd1", rd1)

        def compile(self, ver):
            key = (self.name, ver)
            if (r := _COMPILE_CACHE.get(key)) is not None:
                return r
            from concourse.dve_ops import get_dve_sub_opcode
            result = DveOpSpec(
                name=self.name,
                opcode=get_dve_sub_opcode(self.name),
                uops=self._uops,
                rd1_en=self._rd1,
            )
            _COMPILE_CACHE[key] = result
            return result

    def _reg(op):
        if op.name in _SUB_OPCODE_FOR_NAME:
            return next(o for o in OPS if o.name == op.name)
        row = _CUSTOM_DVE_ROW_BASE + len(OPS)
        assert row < 0x20, "custom DVE row budget exceeded"
        OPS.append(op)
        _SUB_OPCODE_FOR_NAME[op.name] = row
        CUSTOM_DVE_SPECS[op.name] = op.spec
        return op

    fake = Spec(body=(Src0 + C0) * relu(Src0), reference=_burgers_ref)
    return _reg(HandDveOp("BURGERS1_ANT", fake, [_burgers_uop()], False))


OP_FUSED = _register_ops()

# ---------------------------------------------------------------------------
# Kernel build (cached)
# ---------------------------------------------------------------------------

_CACHE = {}

DEFAULT_SCHED = (2048, 4096, 4096, 4096, 4096, 4096, 4096, 4096, 2048)


def build_nc(beta, sched=DEFAULT_SCHED, x_bufs=4, o_bufs=4, cw=0):
    key = (float(beta), tuple(sched), x_bufs, o_bufs, cw)
    if key in _CACHE:
        return _CACHE[key]
    widths = list(sched)
    assert sum(widths) == R, (sum(widths), R)
    f16 = mybir.dt.float16

    nc = bacc.Bacc("TRN2", target_bir_lowering=False, debug=False)
    x = nc.dram_tensor("x", [S + 2], f16, kind="ExternalInput")
    y = nc.dram_tensor("y", [S], f16, kind="ExternalOutput")
    xh = getattr(x, "tensor", x)
    yh = getattr(y, "tensor", y)

    with TileContext(nc) as tc:
        with (
            tc.tile_pool(name="x", bufs=x_bufs) as xp,
            tc.tile_pool(name="o", bufs=o_bufs) as op_,
        ):
            # Loads on the SP HWDGE ring; stores on the ACT HWDGE ring —
            # separate FIFOs, so a store queued behind the next tile's load
            # can't head-of-line block it. beta is baked as an immediate
            # (recompiled per d_org; compile time is host-side only).
            #
            # cw > 0 decouples the DMA tile width G from the compute width:
            # one load/store per G-wide tile (bigger per-partition DMA
            # segments) with ceil(G/cw) DVE sub-ops. Sub-ops are issued in
            # REVERSE offset order: each sub-op's first 2 output columns are
            # stale-swap garbage, and the preceding chunk's valid tail
            # overwrites them (cols a, a+1 are the last 2 outputs of the
            # chunk at a-cw).
            off = 0
            for G in widths:
                t0 = off
                off += G
                xt = xp.tile([P, G + 2], f16, tag="x")
                src = AP(xh, t0, [[R, P], [1, G + 2]])
                nc.sync.dma_start(out=xt[:, :], in_=src)
                ot = op_.tile([P, G + 2], f16, tag="o")
                C = cw if cw else G
                for a in reversed(range(0, G, C)):
                    w = min(C, G - a)
                    nc.vector._custom_dve(OP_FUSED, out=ot[:, a:a + w + 2],
                                          in0=xt[:, a:a + w + 2],
                                          s0=float(beta), s1=0.0)
                dst = AP(yh, t0, [[R, P], [1, G]])
                nc.scalar.dma_start(out=dst, in_=ot[:, 2:G + 2])
    nc.compile()
    _CACHE[key] = nc
    return nc


# ---------------------------------------------------------------------------
# Host entry point
# ---------------------------------------------------------------------------

def _axon_device_reset():
    try:
        import ctypes
        import time as _time
        lib = ctypes.CDLL("/opt/axon/libaxon_pjrt.so")
        lib.axon_reset.restype = ctypes.c_int64
        lib.axon_reset()
        _time.sleep(2.0)
    except Exception:
        pass


def kernel(state, bc, d_org, _trace=False, _build_kwargs=None):
    state = np.asarray(state)
    bc = np.asarray(bc)
    d_org = np.asarray(d_org)
    in_dtype = state.dtype

    flat = state.reshape(-1).astype(np.float32, copy=False)
    bcf = bc.reshape(-1).astype(np.float32)
    d = np.float32(0.01) / (np.float32(1.0) + np.exp(-d_org.astype(np.float32)))

    # y-space: pre-scale by 1/sqrt(DX) = 10; beta = d/DX^1.5 = d*1000
    beta = float(np.float32(d) * np.float32(1000.0))
    nc = build_nc(beta, **(_build_kwargs or {}))

    U = np.empty(N_TOTAL + 2, dtype=np.float16)
    np.multiply(flat, np.float32(10.0), out=U[1:-1])
    U[0] = bcf[0] * np.float32(10.0)
    U[-1] = bcf[1] * np.float32(10.0)

    in_maps = [
        {"x": U[c * S: c * S + S + 2]}
        for c in range(N_CORES)
    ]
    try:
        res = run_bass_kernel_spmd(nc, in_maps, core_ids=list(range(N_CORES)),
                                   trace=_trace)
    except Exception:
        # A prior crash can leave the accelerator wedged; reset and retry once.
        _axon_device_reset()
        res = run_bass_kernel_spmd(nc, in_maps, core_ids=list(range(N_CORES)),
                                   trace=_trace)
    out = np.concatenate([np.asarray(res.results[c]["y"])
                          for c in range(N_CORES)])
    out = out.astype(np.float32).reshape(1, 1, N_TOTAL).astype(
        in_dtype, copy=False)
    if _trace:
        return out, res
    return out
